# revision 5
# baseline (speedup 1.0000x reference)
"""Sobel filter Trainium2 Bass kernel.

Problem: img [32, 3, 512, 512] f32, kx/ky [1, 3, 3, 3] f32 (same 3x3 kernel
broadcast over the 3 input channels in the reference, but we honor arbitrary
values). Output [32, 1, 512, 512] f32:
    Gx = valid_conv3x3(img, kx), Gy = valid_conv3x3(img, ky)  -> [N,1,510,510]
    out = sqrt(Gx^2 + Gy^2) edge-padded by 1 back to [N,1,512,512]

Strategy (pure data parallel over 8 NeuronCores, 4 images per core):
  The reference Sobel kernels are rank-1 (channel-proportional and separable):
  kG[c, dy, dx] = a[c] * b[dy] * g[dx].  Then
      G = Xconv_g( Sum_c a[c] * Yconv_b(img_c) )
  The y-conv + channel sum runs on the TensorEngine as 3 PSUM-accumulated
  banded matmuls per group (stationary = banded [K=128, M] matrix built from
  a[c]*b on host), the 3-tap x-conv + magnitude runs as a short bf16
  elementwise chain balanced across DVE / ACT / GpSimd.

  Row tiling per image (out rows 0..511):
    tile0:  img rows   0:128 -> out rows   0:127 (127 rows; stationary col 0
            duplicates y'=0 for the top edge pad)
    tile1-3: img rows 126t:126t+128 -> out rows 126t+1 : 126t+127 (126 rows)
    mini:   img rows 504:512 of ALL 4 images ([32, ...] block-diagonal
            stationary) -> out rows 505:512 (7 rows/image; last col of each
            block duplicates y'=509 for the bottom edge pad)
  Each big tile is ONE 3-channel load DMA ([128, 3, 512], 2KB lines) and ONE
  store DMA.  Magnitude is computed and stored in bf16 (rel err ~6e-3, host
  upcasts to f32); this halves store traffic and doubles DVE throughput.

  Epilogue per tile (all work tiles bf16):
    ACT:    cp1 = copy(ps1); sqy = square(gy); edge cols; sqrt; store trigger
    GpSimd: cp2 = copy(ps2); s = sqx + sqy
    DVE:    gx = cp1_0 - cp1_2; gyt = 0.5*cp2_0 + cp2_1;
            gy = 0.5*cp2_2 + gyt; sqx = gx*gx
    Sync:   load triggers
  Both ACT tables (SQUARE, SQRT) are primed by dummy ops at kernel start so
  the lazy table loads don't sit on the critical path.

The general (non-rank-1) fallback keeps the baseline 18-matmul path.
"""

import os

import numpy as np

N_CORES = 8
N_FULL = 32          # full batch
N_PER_CORE = N_FULL // N_CORES
H = W = 512
TILE_K = 128         # input rows per full row-tile
NW = 510             # valid output columns
STAT_M = 127         # stationary cols: dup-edge col + band offsets 0..125
MINI_K = 8 * N_PER_CORE   # 4 images x 8 input rows
MINI_M = 7 * N_PER_CORE   # 4 images x 7 output rows (incl. bottom edge dup)

# big-tile row plan: (img_row0, out_row0, out_rows, stat_col0)
BIG_TILES = [
    (0, 0, 127, 0),
    (126, 127, 126, 1),
    (252, 253, 126, 1),
    (378, 379, 126, 1),
]

_CACHE: dict = {}
LAST_RESULTS = None  # BassKernelResults of the most recent run (for test.py)


# ---------------------------------------------------------------------------
# General fallback (arbitrary kx/ky): banded-Toeplitz matmuls per (g, c, dx).
# Kept from the baseline kernel; only used when the kernels are not rank-1.
# ---------------------------------------------------------------------------

GEN_TILE_M = 126
GEN_N_TILES = 4
GEN_MINI_K = 8 * N_PER_CORE
GEN_MINI_M = 6 * N_PER_CORE


def _build_stationaries(kx: np.ndarray, ky: np.ndarray):
    ks = (np.asarray(kx, np.float32), np.asarray(ky, np.float32))
    stat = np.zeros((18, TILE_K, GEN_TILE_M), np.float32)
    mini = np.zeros((18, GEN_MINI_K, GEN_MINI_M), np.float32)
    m = np.arange(GEN_TILE_M)
    mm = np.arange(6)
    i = 0
    for g in range(2):
        for c in range(3):
            for dx in range(3):
                for dy in range(3):
                    stat[i, m + dy, m] = ks[g][0, c, dy, dx]
                    for j in range(N_PER_CORE):
                        mini[i, j * 8 + mm + dy, j * 6 + mm] = ks[g][0, c, dy, dx]
                i += 1
    return (
        np.ascontiguousarray(stat.transpose(1, 0, 2)),
        np.ascontiguousarray(mini.transpose(1, 0, 2)),
    )


def _epilogue_gen(nc, work_pool, psx, psy, rows, f32):
    s = work_pool.tile([rows, W], f32, tag="s", name="s")
    s2 = work_pool.tile([rows, NW], f32, tag="s2", name="s2")
    nc.scalar.square(s[:, 1 : 1 + NW], psx)
    nc.scalar.square(s2, psy)
    nc.vector.tensor_add(s[:, 1 : 1 + NW], s[:, 1 : 1 + NW], s2)
    nc.vector.tensor_copy(s[:, 0:1], s[:, 1:2])
    nc.vector.tensor_copy(s[:, W - 1 : W], s[:, W - 2 : W - 1])
    mag = work_pool.tile([rows, W], f32, tag="mag", name="mag")
    nc.scalar.sqrt(mag, s)
    return mag


def _sobel_body_gen(tc, out, img, stat_dram, stat_mini_dram):
    import concourse.mybir as mybir

    nc = tc.nc
    f32 = mybir.dt.float32
    mm_dt = mybir.dt.float32r

    img_yx = img.rearrange("n c y x -> n y c x")

    with (
        tc.tile_pool(name="const", bufs=1) as const_pool,
        tc.tile_pool(name="imgs", bufs=3) as img_pool,
        tc.tile_pool(name="work", bufs=4) as work_pool,
        tc.tile_pool(name="psum", bufs=2, space="PSUM") as psum_pool,
    ):
        stat_mini_sb = const_pool.tile([GEN_MINI_K, 18, GEN_MINI_M], mm_dt)
        nc.sync.dma_start(out=stat_mini_sb, in_=stat_mini_dram)
        mit = img_pool.tile([GEN_MINI_K, 3, W], mm_dt, tag="mit", bufs=1)
        for c in range(3):
            nc.sync.dma_start(out=mit[:, c, :], in_=img_yx[:, H - 8 : H, c])
        stat_sb = const_pool.tile([TILE_K, 18, GEN_TILE_M], mm_dt)
        for j in range(5):
            nc.sync.dma_start(
                out=stat_sb[:, 2 * j : 2 * j + 2], in_=stat_dram[:, 2 * j : 2 * j + 2]
            )
        for j in range(5, 9):
            nc.scalar.dma_start(
                out=stat_sb[:, 2 * j : 2 * j + 2], in_=stat_dram[:, 2 * j : 2 * j + 2]
            )

        def big_tile(n, t):
            y0 = t * GEN_TILE_M
            its = []
            for c in range(3):
                itc = img_pool.tile(
                    [TILE_K, W], mm_dt, tag=f"it{c}", name=f"it{c}", bufs=6
                )
                nc.sync.dma_start(out=itc, in_=img_yx[n, y0 : y0 + TILE_K, c])
                its.append(itc)

            psx = psum_pool.tile([GEN_TILE_M, NW], f32, tag="psx", name="psx")
            psy = psum_pool.tile([GEN_TILE_M, NW], f32, tag="psy", name="psy")
            for g, ps in ((0, psx), (1, psy)):
                mmi = 0
                for c in range(3):
                    for dx in range(3):
                        i = (g * 3 + c) * 3 + dx
                        nc.tensor.matmul(
                            ps,
                            stat_sb[:, i, :],
                            its[c][:, dx : dx + NW],
                            start=(mmi == 0),
                            stop=(mmi == 8),
                        )
                        mmi += 1

            mag = _epilogue_gen(nc, work_pool, psx, psy, GEN_TILE_M, f32)
            nc.scalar.dma_start(out=out[n, 1 + y0 : 1 + y0 + GEN_TILE_M, :], in_=mag)
            if t == 0:
                nc.scalar.dma_start(out=out[n, 0:1, :], in_=mag[0:1, :])

        def mini_tile():
            mpsx = psum_pool.tile([GEN_MINI_M, NW], f32, tag="mpsx", bufs=1, name="mpsx")
            mpsy = psum_pool.tile([GEN_MINI_M, NW], f32, tag="mpsy", bufs=1, name="mpsy")
            for g, ps in ((0, mpsx), (1, mpsy)):
                mmi = 0
                for c in range(3):
                    for dx in range(3):
                        i = (g * 3 + c) * 3 + dx
                        nc.tensor.matmul(
                            ps,
                            stat_mini_sb[:, i, :],
                            mit[:, c, dx : dx + NW],
                            start=(mmi == 0),
                            stop=(mmi == 8),
                        )
                        mmi += 1
            mmag = _epilogue_gen(nc, work_pool, mpsx, mpsy, GEN_MINI_M, f32)
            for n in range(N_PER_CORE):
                nc.scalar.dma_start(
                    out=out[n, H - 7 : H - 1, :], in_=mmag[n * 6 : n * 6 + 6]
                )
                nc.scalar.dma_start(
                    out=out[n, H - 1 : H, :], in_=mmag[n * 6 + 5 : n * 6 + 6]
                )

        mini_tile()
        for n in range(N_PER_CORE):
            for t in range(GEN_N_TILES):
                big_tile(n, t)


def _build_program_gen():
    import concourse.bacc as bacc
    import concourse.mybir as mybir
    import concourse.tile as tile

    nc = bacc.Bacc(
        "TRN2",
        target_bir_lowering=False,
        debug=False,
        num_devices=N_CORES,
    )
    img = nc.dram_tensor(
        "img", [N_PER_CORE, 3, H, W], mybir.dt.float32r, kind="ExternalInput"
    ).ap()
    stat = nc.dram_tensor(
        "stat", [TILE_K, 18, GEN_TILE_M], mybir.dt.float32r, kind="ExternalInput"
    ).ap()
    stat_mini = nc.dram_tensor(
        "stat_mini", [GEN_MINI_K, 18, GEN_MINI_M], mybir.dt.float32r,
        kind="ExternalInput",
    ).ap()
    out = nc.dram_tensor(
        "out", [N_PER_CORE, H, W], mybir.dt.float32, kind="ExternalOutput"
    ).ap()

    with tile.TileContext(nc) as tc:
        _sobel_body_gen(tc, out, img, stat, stat_mini)
    nc.compile()
    return nc


# ---------------------------------------------------------------------------
# Separable fast path.
# ---------------------------------------------------------------------------


def _rank1_decompose(k: np.ndarray):
    """k [1,3,3,3] -> (a[3], b[3], g[3]) with k[0,c,dy,dx] = a_c b_dy g_dx,
    or None if not (numerically exactly) rank-1."""
    k2 = np.asarray(k, np.float64)[0]
    scale = np.abs(k2).max()
    if scale == 0:
        return None
    u, s, vt = np.linalg.svd(k2.reshape(3, 9), full_matrices=False)
    a = u[:, 0] * s[0]
    v = vt[0].reshape(3, 3)
    u2, s2, vt2 = np.linalg.svd(v, full_matrices=False)
    b = u2[:, 0] * s2[0]
    g = vt2[0]
    rec = np.einsum("c,y,x->cyx", a, b, g)
    if np.abs(rec - k2).max() > 1e-6 * scale:
        return None
    # normalize so the largest |g| tap is exactly 1
    gm = g[np.argmax(np.abs(g))]
    g = g / gm
    a = a * gm
    return a.astype(np.float64), b.astype(np.float64), g.astype(np.float64)


def _build_stationaries_sep(ax, bx, ay, by):
    """stat_sep [TILE_K, 6, STAT_M] (j = G*3+c): col 0 duplicates the y'=0
    band (top edge pad), cols 1..126 are band offsets 0..125.
    stat_sep_mini [MINI_K, 6, MINI_M]: block-diagonal per image, 7 out cols
    each (col 6 duplicates y'=509 for the bottom edge pad)."""
    stat = np.zeros((6, TILE_K, STAT_M), np.float32)
    mini = np.zeros((6, MINI_K, MINI_M), np.float32)
    m = np.arange(126)
    mm = np.arange(6)
    for gi, (a, b) in enumerate(((ax, bx), (ay, by))):
        for c in range(3):
            j = gi * 3 + c
            for dy in range(3):
                w = np.float32(a[c] * b[dy])
                stat[j, dy, 0] = w              # dup col: y'=0
                stat[j, m + dy, m + 1] = w      # band offsets 0..125
                for im in range(N_PER_CORE):
                    mini[j, im * 8 + mm + dy, im * 7 + mm] = w
                    mini[j, im * 8 + 5 + dy, im * 7 + 6] = w  # dup col: y'=509
    return (
        np.ascontiguousarray(stat.transpose(1, 0, 2)),
        np.ascontiguousarray(mini.transpose(1, 0, 2)),
    )


def _emit_xconv_sbuf(nc, cp, taps, outt, gyt, r):
    """outt[:r] = x-conv of SBUF tile cp[:r] with 3-tap `taps`.
    The overall sign is free (the result gets squared), so tap patterns are
    normalized to hit the cheap forms: +-1 pairs -> one tensor add/sub;
    mid-tap +-1 triples -> two scalar_tensor_tensor ops (via gyt scratch)."""
    import concourse.mybir as mybir

    L = [(float(taps[dx]), dx) for dx in range(3) if taps[dx] != 0.0]
    assert L
    mult, add = mybir.AluOpType.mult, mybir.AluOpType.add
    if len(L) == 1:
        w, dx = L[0]
        nc.vector.tensor_scalar_mul(outt[:r], cp[:r, dx : dx + NW], w)
    elif len(L) == 2 and abs(L[0][0]) == 1.0 and abs(L[1][0]) == 1.0:
        s0 = cp[:r, L[0][1] : L[0][1] + NW]
        s1 = cp[:r, L[1][1] : L[1][1] + NW]
        if L[0][0] * L[1][0] < 0:
            nc.vector.tensor_sub(outt[:r], s0, s1)
        else:
            nc.vector.tensor_add(outt[:r], s0, s1)
    elif len(L) == 3 and abs(L[1][0]) == 1.0:
        sgn = L[1][0]  # flip all taps so the middle tap is +1 (sign is free)
        (w0, d0), (_, d1), (w2, d2) = L
        nc.vector.scalar_tensor_tensor(
            gyt[:r], cp[:r, d0 : d0 + NW], w0 * sgn, cp[:r, d1 : d1 + NW], mult, add
        )
        nc.vector.scalar_tensor_tensor(
            outt[:r], cp[:r, d2 : d2 + NW], w2 * sgn, gyt[:r], mult, add
        )
    else:
        acc = None
        for w, dx in L:
            src = cp[:r, dx : dx + NW]
            if acc is None:
                nc.vector.tensor_scalar_mul(outt[:r], src, w)
            else:
                nc.vector.scalar_tensor_tensor(outt[:r], src, w, acc, mult, add)
            acc = outt


def _sobel_body_sep(tc, out, img, stat_dram, stat_mini_dram, gx_taps, gy_taps):
    import concourse.mybir as mybir

    nc = tc.nc
    f32 = mybir.dt.float32
    bf16 = mybir.dt.bfloat16
    mm_dt = mybir.dt.float32r

    img_yx = img.rearrange("n c y x -> n y c x")

    with (
        tc.tile_pool(name="const", bufs=1) as const_pool,
        tc.tile_pool(name="imgs", bufs=5) as img_pool,
        tc.tile_pool(name="work", bufs=3) as work_pool,
        tc.tile_pool(name="psum", bufs=3, space="PSUM") as psum_pool,
    ):
        # Prime both ACT tables (SQUARE, SQRT) while the first DMAs stream so
        # the 1.3us lazy table loads stay off the critical path.
        dmy = const_pool.tile([1, 4], f32)
        nc.vector.memset(dmy[:, 0:2], 1.0)
        nc.scalar.square(dmy[:, 2:3], dmy[:, 0:1])
        nc.scalar.sqrt(dmy[:, 3:4], dmy[:, 1:2])

        stat_mini_sb = const_pool.tile([MINI_K, 6, MINI_M], mm_dt)
        nc.sync.dma_start(out=stat_mini_sb, in_=stat_mini_dram)
        mit = img_pool.tile([MINI_K, 3, W], mm_dt, tag="mit", bufs=1)
        for c in range(3):
            nc.sync.dma_start(out=mit[:, c, :], in_=img_yx[:, H - 8 : H, c])
        stat_sb = const_pool.tile([TILE_K, 6, STAT_M], mm_dt)
        nc.sync.dma_start(out=stat_sb, in_=stat_dram)

        def epilogue(ps1, ps2, r):
            """bf16 x-conv + magnitude from the two y-conv PSUM tiles."""
            cp1 = work_pool.tile([TILE_K, W], bf16, tag="cp1", name="cp1")
            nc.scalar.copy(cp1[:r], ps1[:r])
            cp2 = work_pool.tile([TILE_K, W], bf16, tag="cp2", name="cp2")
            nc.vector.tensor_copy(cp2[:r], ps2[:r])

            gx = work_pool.tile([TILE_K, NW], bf16, tag="gx", name="gx")
            gyt = work_pool.tile([TILE_K, NW], bf16, tag="gyt", name="gyt")
            gy = work_pool.tile([TILE_K, NW], bf16, tag="gy", name="gy")
            _emit_xconv_sbuf(nc, cp1, gx_taps, gx, gyt, r)
            _emit_xconv_sbuf(nc, cp2, gy_taps, gy, gyt, r)

            sqx = work_pool.tile([TILE_K, NW], bf16, tag="sqx", name="sqx")
            nc.gpsimd.tensor_mul(sqx[:r], gx[:r], gx[:r])
            sqy = work_pool.tile([TILE_K, NW], bf16, tag="sqy", name="sqy")
            nc.scalar.square(sqy[:r], gy[:r])
            s = work_pool.tile([TILE_K, W], bf16, tag="s", name="s")
            nc.gpsimd.tensor_add(s[:r, 1 : 1 + NW], sqx[:r], sqy[:r])
            nc.scalar.copy(s[:r, 0:1], s[:r, 1:2])
            nc.scalar.copy(s[:r, W - 1 : W], s[:r, W - 2 : W - 1])
            mag = work_pool.tile([TILE_K, W], bf16, tag="mag", name="mag")
            nc.scalar.sqrt(mag[:r], s[:r])
            return mag

        # mini tile first (tiny deps -> PE starts early and ramps its clock)
        mps1 = psum_pool.tile([MINI_M, W], f32, tag="mps1", bufs=1, name="mps1")
        mps2 = psum_pool.tile([MINI_M, W], f32, tag="mps2", bufs=1, name="mps2")
        for gi, ps in ((0, mps1), (1, mps2)):
            for c in range(3):
                nc.tensor.matmul(
                    ps,
                    stat_mini_sb[:, gi * 3 + c, :],
                    mit[:, c, :],
                    start=(c == 0),
                    stop=(c == 2),
                )
        mmag = epilogue(mps1, mps2, MINI_M)
        nc.scalar.dma_start(out=out[:, H - 7 : H, :], in_=mmag[:MINI_M])

        for n in range(N_PER_CORE):
            for (y0, o0, r, sc0) in BIG_TILES:
                it = img_pool.tile(
                    [TILE_K, 3, W], mm_dt, tag="it", name="it", bufs=5
                )
                nc.sync.dma_start(out=it, in_=img_yx[n, y0 : y0 + TILE_K])
                ps1 = psum_pool.tile([STAT_M, W], f32, tag="ps1", name="ps1")
                ps2 = psum_pool.tile([STAT_M, W], f32, tag="ps2", name="ps2")
                for gi, ps in ((0, ps1), (1, ps2)):
                    for c in range(3):
                        nc.tensor.matmul(
                            ps[:r],
                            stat_sb[:, gi * 3 + c, sc0 : sc0 + r],
                            it[:, c, :],
                            start=(c == 0),
                            stop=(c == 2),
                        )
                mag = epilogue(ps1, ps2, r)
                nc.scalar.dma_start(out=out[n, o0 : o0 + r, :], in_=mag[:r])


def _build_program_sep(gx_taps, gy_taps):
    import concourse.bacc as bacc
    import concourse.mybir as mybir
    import concourse.tile as tile

    nc = bacc.Bacc(
        "TRN2", target_bir_lowering=False, debug=False, num_devices=N_CORES
    )
    img = nc.dram_tensor(
        "img", [N_PER_CORE, 3, H, W], mybir.dt.float32r, kind="ExternalInput"
    ).ap()
    stat = nc.dram_tensor(
        "stat", [TILE_K, 6, STAT_M], mybir.dt.float32r, kind="ExternalInput"
    ).ap()
    stat_mini = nc.dram_tensor(
        "stat_mini", [MINI_K, 6, MINI_M], mybir.dt.float32r, kind="ExternalInput"
    ).ap()
    out = nc.dram_tensor(
        "out", [N_PER_CORE, H, W], mybir.dt.bfloat16, kind="ExternalOutput"
    ).ap()
    with tile.TileContext(nc) as tc:
        _sobel_body_sep(tc, out, img, stat, stat_mini, gx_taps, gy_taps)
    nc.compile()
    return nc


def _run(nc, in_maps, out_bf16):
    global LAST_RESULTS
    from concourse.bass_utils import run_bass_kernel_spmd

    trace = os.environ.get("SOBEL_TRACE", "0") == "1"
    res = run_bass_kernel_spmd(
        nc, in_maps, core_ids=list(range(N_CORES)), trace=trace
    )
    LAST_RESULTS = res
    outs = [np.asarray(res.results[c]["out"]) for c in range(N_CORES)]
    if out_bf16:
        outs = [o.astype(np.float32) for o in outs]
    out = np.concatenate(outs, axis=0)
    return np.ascontiguousarray(out.reshape(N_FULL, 1, H, W))


def kernel(img: np.ndarray, kx: np.ndarray, ky: np.ndarray) -> np.ndarray:
    img = np.ascontiguousarray(np.asarray(img, dtype=np.float32))
    assert img.shape == (N_FULL, 3, H, W), img.shape

    dx_ = _rank1_decompose(kx) if os.environ.get("SOBEL_NO_SEP", "0") != "1" else None
    dy_ = _rank1_decompose(ky) if dx_ is not None else None
    if dx_ is not None and dy_ is not None:
        (axc, bx, gx_t), (ayc, by, gy_t) = dx_, dy_
        stat, stat_mini = _build_stationaries_sep(axc, bx, ayc, by)
        key = ("sep", tuple(np.round(gx_t, 12)), tuple(np.round(gy_t, 12)))
        if key not in _CACHE:
            _CACHE[key] = _build_program_sep(tuple(gx_t), tuple(gy_t))
        nc = _CACHE[key]
        out_bf16 = True
    else:
        stat, stat_mini = _build_stationaries(kx, ky)
        if "gen" not in _CACHE:
            _CACHE["gen"] = _build_program_gen()
        nc = _CACHE["gen"]
        out_bf16 = False

    in_maps = [
        {
            "img": img[c * N_PER_CORE : (c + 1) * N_PER_CORE],
            "stat": stat,
            "stat_mini": stat_mini,
        }
        for c in range(N_CORES)
    ]
    return _run(nc, in_maps, out_bf16)


# revision 9
# speedup vs baseline: 1.0579x; 1.0579x over previous
"""Sobel filter Trainium2 Bass kernel.

Problem: img [32, 3, 512, 512] f32, kx/ky [1, 3, 3, 3] f32 (same 3x3 kernel
broadcast over the 3 input channels in the reference, but we honor arbitrary
values). Output [32, 1, 512, 512] f32:
    Gx = valid_conv3x3(img, kx), Gy = valid_conv3x3(img, ky)  -> [N,1,510,510]
    out = sqrt(Gx^2 + Gy^2) edge-padded by 1 back to [N,1,512,512]

Strategy (pure data parallel over 8 NeuronCores, 4 images per core):
  The reference Sobel kernels are rank-1 (channel-proportional and separable):
  kG[c, dy, dx] = a[c] * b[dy] * g[dx].  Then
      G = Xconv_g( Sum_c a[c] * Yconv_b(img_c) )
  The y-conv + channel sum runs on the TensorEngine as 3 PSUM-accumulated
  banded matmuls per group (stationary = banded [K=128, M] matrix built from
  a[c]*b on host), the 3-tap x-conv + magnitude runs as a short bf16
  elementwise chain balanced across DVE / ACT / GpSimd.

  Row tiling per image (out rows 0..511):
    tile0:  img rows   0:128 -> out rows   0:127 (127 rows; stationary col 0
            duplicates y'=0 for the top edge pad)
    tile1-3: img rows 126t:126t+128 -> out rows 126t+1 : 126t+127 (126 rows)
    mini:   img rows 504:512 of ALL 4 images ([32, ...] block-diagonal
            stationary) -> out rows 505:512 (7 rows/image; last col of each
            block duplicates y'=509 for the bottom edge pad)
  Each big tile is ONE 3-channel load DMA ([128, 3, 512], 2KB lines) and ONE
  store DMA.  Magnitude is computed and stored in bf16 (rel err ~6e-3, host
  upcasts to f32); this halves store traffic and doubles DVE throughput.

  Epilogue per tile (all work tiles bf16):
    ACT:    cp1 = copy(ps1); sqy = square(gy); edge cols; sqrt; store trigger
    GpSimd: cp2 = copy(ps2); s = sqx + sqy
    DVE:    gx = cp1_0 - cp1_2; gyt = 0.5*cp2_0 + cp2_1;
            gy = 0.5*cp2_2 + gyt; sqx = gx*gx
    Sync:   load triggers
  Both ACT tables (SQUARE, SQRT) are primed by dummy ops at kernel start so
  the lazy table loads don't sit on the critical path.

The general (non-rank-1) fallback keeps the baseline 18-matmul path.
"""

import os

import numpy as np

N_CORES = 8
N_FULL = 32          # full batch
N_PER_CORE = N_FULL // N_CORES
H = W = 512
TILE_K = 128         # input rows per full row-tile
NW = 510             # valid output columns
STAT_M = 127         # stationary cols: dup-edge col + band offsets 0..125
MINI_K = 8 * N_PER_CORE   # 4 images x 8 input rows
MINI_M = 7 * N_PER_CORE   # 4 images x 7 output rows (incl. bottom edge dup)

# big-tile row plan: (img_row0, out_row0, out_rows, stat_col0)
BIG_TILES = [
    (0, 0, 127, 0),
    (126, 127, 126, 1),
    (252, 253, 126, 1),
    (378, 379, 126, 1),
]

_CACHE: dict = {}
LAST_RESULTS = None  # BassKernelResults of the most recent run (for test.py)


# ---------------------------------------------------------------------------
# General fallback (arbitrary kx/ky): banded-Toeplitz matmuls per (g, c, dx).
# Kept from the baseline kernel; only used when the kernels are not rank-1.
# ---------------------------------------------------------------------------

GEN_TILE_M = 126
GEN_N_TILES = 4
GEN_MINI_K = 8 * N_PER_CORE
GEN_MINI_M = 6 * N_PER_CORE


def _build_stationaries(kx: np.ndarray, ky: np.ndarray):
    ks = (np.asarray(kx, np.float32), np.asarray(ky, np.float32))
    stat = np.zeros((18, TILE_K, GEN_TILE_M), np.float32)
    mini = np.zeros((18, GEN_MINI_K, GEN_MINI_M), np.float32)
    m = np.arange(GEN_TILE_M)
    mm = np.arange(6)
    i = 0
    for g in range(2):
        for c in range(3):
            for dx in range(3):
                for dy in range(3):
                    stat[i, m + dy, m] = ks[g][0, c, dy, dx]
                    for j in range(N_PER_CORE):
                        mini[i, j * 8 + mm + dy, j * 6 + mm] = ks[g][0, c, dy, dx]
                i += 1
    return (
        np.ascontiguousarray(stat.transpose(1, 0, 2)),
        np.ascontiguousarray(mini.transpose(1, 0, 2)),
    )


def _epilogue_gen(nc, work_pool, psx, psy, rows, f32):
    s = work_pool.tile([rows, W], f32, tag="s", name="s")
    s2 = work_pool.tile([rows, NW], f32, tag="s2", name="s2")
    nc.scalar.square(s[:, 1 : 1 + NW], psx)
    nc.scalar.square(s2, psy)
    nc.vector.tensor_add(s[:, 1 : 1 + NW], s[:, 1 : 1 + NW], s2)
    nc.vector.tensor_copy(s[:, 0:1], s[:, 1:2])
    nc.vector.tensor_copy(s[:, W - 1 : W], s[:, W - 2 : W - 1])
    mag = work_pool.tile([rows, W], f32, tag="mag", name="mag")
    nc.scalar.sqrt(mag, s)
    return mag


def _sobel_body_gen(tc, out, img, stat_dram, stat_mini_dram):
    import concourse.mybir as mybir

    nc = tc.nc
    f32 = mybir.dt.float32
    mm_dt = mybir.dt.float32r

    img_yx = img.rearrange("n c y x -> n y c x")

    with (
        tc.tile_pool(name="const", bufs=1) as const_pool,
        tc.tile_pool(name="imgs", bufs=3) as img_pool,
        tc.tile_pool(name="work", bufs=4) as work_pool,
        tc.tile_pool(name="psum", bufs=2, space="PSUM") as psum_pool,
    ):
        stat_mini_sb = const_pool.tile([GEN_MINI_K, 18, GEN_MINI_M], mm_dt)
        nc.sync.dma_start(out=stat_mini_sb, in_=stat_mini_dram)
        mit = img_pool.tile([GEN_MINI_K, 3, W], mm_dt, tag="mit", bufs=1)
        for c in range(3):
            nc.sync.dma_start(out=mit[:, c, :], in_=img_yx[:, H - 8 : H, c])
        stat_sb = const_pool.tile([TILE_K, 18, GEN_TILE_M], mm_dt)
        for j in range(5):
            nc.sync.dma_start(
                out=stat_sb[:, 2 * j : 2 * j + 2], in_=stat_dram[:, 2 * j : 2 * j + 2]
            )
        for j in range(5, 9):
            nc.scalar.dma_start(
                out=stat_sb[:, 2 * j : 2 * j + 2], in_=stat_dram[:, 2 * j : 2 * j + 2]
            )

        def big_tile(n, t):
            y0 = t * GEN_TILE_M
            its = []
            for c in range(3):
                itc = img_pool.tile(
                    [TILE_K, W], mm_dt, tag=f"it{c}", name=f"it{c}", bufs=6
                )
                nc.sync.dma_start(out=itc, in_=img_yx[n, y0 : y0 + TILE_K, c])
                its.append(itc)

            psx = psum_pool.tile([GEN_TILE_M, NW], f32, tag="psx", name="psx")
            psy = psum_pool.tile([GEN_TILE_M, NW], f32, tag="psy", name="psy")
            for g, ps in ((0, psx), (1, psy)):
                mmi = 0
                for c in range(3):
                    for dx in range(3):
                        i = (g * 3 + c) * 3 + dx
                        nc.tensor.matmul(
                            ps,
                            stat_sb[:, i, :],
                            its[c][:, dx : dx + NW],
                            start=(mmi == 0),
                            stop=(mmi == 8),
                        )
                        mmi += 1

            mag = _epilogue_gen(nc, work_pool, psx, psy, GEN_TILE_M, f32)
            nc.scalar.dma_start(out=out[n, 1 + y0 : 1 + y0 + GEN_TILE_M, :], in_=mag)
            if t == 0:
                nc.scalar.dma_start(out=out[n, 0:1, :], in_=mag[0:1, :])

        def mini_tile():
            mpsx = psum_pool.tile([GEN_MINI_M, NW], f32, tag="mpsx", bufs=1, name="mpsx")
            mpsy = psum_pool.tile([GEN_MINI_M, NW], f32, tag="mpsy", bufs=1, name="mpsy")
            for g, ps in ((0, mpsx), (1, mpsy)):
                mmi = 0
                for c in range(3):
                    for dx in range(3):
                        i = (g * 3 + c) * 3 + dx
                        nc.tensor.matmul(
                            ps,
                            stat_mini_sb[:, i, :],
                            mit[:, c, dx : dx + NW],
                            start=(mmi == 0),
                            stop=(mmi == 8),
                        )
                        mmi += 1
            mmag = _epilogue_gen(nc, work_pool, mpsx, mpsy, GEN_MINI_M, f32)
            for n in range(N_PER_CORE):
                nc.scalar.dma_start(
                    out=out[n, H - 7 : H - 1, :], in_=mmag[n * 6 : n * 6 + 6]
                )
                nc.scalar.dma_start(
                    out=out[n, H - 1 : H, :], in_=mmag[n * 6 + 5 : n * 6 + 6]
                )

        mini_tile()
        for n in range(N_PER_CORE):
            for t in range(GEN_N_TILES):
                big_tile(n, t)


def _build_program_gen():
    import concourse.bacc as bacc
    import concourse.mybir as mybir
    import concourse.tile as tile

    nc = bacc.Bacc(
        "TRN2",
        target_bir_lowering=False,
        debug=False,
        num_devices=N_CORES,
    )
    img = nc.dram_tensor(
        "img", [N_PER_CORE, 3, H, W], mybir.dt.float32r, kind="ExternalInput"
    ).ap()
    stat = nc.dram_tensor(
        "stat", [TILE_K, 18, GEN_TILE_M], mybir.dt.float32r, kind="ExternalInput"
    ).ap()
    stat_mini = nc.dram_tensor(
        "stat_mini", [GEN_MINI_K, 18, GEN_MINI_M], mybir.dt.float32r,
        kind="ExternalInput",
    ).ap()
    out = nc.dram_tensor(
        "out", [N_PER_CORE, H, W], mybir.dt.float32, kind="ExternalOutput"
    ).ap()

    with tile.TileContext(nc) as tc:
        _sobel_body_gen(tc, out, img, stat, stat_mini)
    nc.compile()
    return nc


# ---------------------------------------------------------------------------
# Separable fast path.
# ---------------------------------------------------------------------------


def _rank1_decompose(k: np.ndarray):
    """k [1,3,3,3] -> (a[3], b[3], g[3]) with k[0,c,dy,dx] = a_c b_dy g_dx,
    or None if not (numerically exactly) rank-1."""
    k2 = np.asarray(k, np.float64)[0]
    scale = np.abs(k2).max()
    if scale == 0:
        return None
    u, s, vt = np.linalg.svd(k2.reshape(3, 9), full_matrices=False)
    a = u[:, 0] * s[0]
    v = vt[0].reshape(3, 3)
    u2, s2, vt2 = np.linalg.svd(v, full_matrices=False)
    b = u2[:, 0] * s2[0]
    g = vt2[0]
    rec = np.einsum("c,y,x->cyx", a, b, g)
    if np.abs(rec - k2).max() > 1e-6 * scale:
        return None
    # normalize so the largest |g| tap is exactly 1
    gm = g[np.argmax(np.abs(g))]
    g = g / gm
    a = a * gm
    return a.astype(np.float64), b.astype(np.float64), g.astype(np.float64)


def _build_stationaries_sep(ax, bx, ay, by):
    """stat_sep [TILE_K, 6, STAT_M] (j = G*3+c): col 0 duplicates the y'=0
    band (top edge pad), cols 1..126 are band offsets 0..125.
    stat_sep_mini [MINI_K, 6, MINI_M]: block-diagonal per image, 7 out cols
    each (col 6 duplicates y'=509 for the bottom edge pad)."""
    stat = np.zeros((6, TILE_K, STAT_M), np.float32)
    mini = np.zeros((6, MINI_K, MINI_M), np.float32)
    m = np.arange(126)
    mm = np.arange(6)
    for gi, (a, b) in enumerate(((ax, bx), (ay, by))):
        for c in range(3):
            j = gi * 3 + c
            for dy in range(3):
                w = np.float32(a[c] * b[dy])
                stat[j, dy, 0] = w              # dup col: y'=0
                stat[j, m + dy, m + 1] = w      # band offsets 0..125
                for im in range(N_PER_CORE):
                    mini[j, im * 8 + mm + dy, im * 7 + mm] = w
                    mini[j, im * 8 + 5 + dy, im * 7 + 6] = w  # dup col: y'=509
    return (
        np.ascontiguousarray(stat.transpose(1, 0, 2)),
        np.ascontiguousarray(mini.transpose(1, 0, 2)),
    )


def _emit_xconv_psum(nc, ps, taps, outt, r, first_on_act=False):
    """outt[:r] = x-conv of PSUM tile ps[:r] with 3-tap `taps`, chained
    in place (each op reads one PSUM slice + the SBUF accumulator).
    Optionally the first (scale-copy) op goes on ACT to offload DVE."""
    import concourse.mybir as mybir

    L = [(float(taps[dx]), dx) for dx in range(3) if taps[dx] != 0.0]
    assert L
    mult, add = mybir.AluOpType.mult, mybir.AluOpType.add
    acc = None
    for w, dx in L:
        src = ps[:r, dx : dx + NW]
        if acc is None:
            if first_on_act and len(L) > 1:
                nc.scalar.mul(outt[:r], src, w)
            else:
                nc.vector.tensor_scalar_mul(outt[:r], src, w)
        else:
            nc.vector.scalar_tensor_tensor(outt[:r], src, w, acc[:r], mult, add)
        acc = outt


def _sobel_body_sep(tc, out, img, stat_dram, stat_mini_dram, gx_taps, gy_taps):
    import concourse.mybir as mybir

    nc = tc.nc
    f32 = mybir.dt.float32
    bf16 = mybir.dt.bfloat16
    mm_dt = mybir.dt.float32r

    img_yx = img.rearrange("n c y x -> n y c x")

    with (
        tc.tile_pool(name="const", bufs=1) as const_pool,
        tc.tile_pool(name="imgs", bufs=5) as img_pool,
        tc.tile_pool(name="work", bufs=3) as work_pool,
        tc.tile_pool(name="psum", bufs=3, space="PSUM") as psum_pool,
    ):
        # Prime both ACT tables (SQUARE, SQRT) while the first DMAs stream so
        # the 1.3us lazy table loads stay off the critical path.
        dmy = const_pool.tile([1, 4], f32)
        nc.vector.memset(dmy[:, 0:2], 1.0)
        nc.scalar.square(dmy[:, 2:3], dmy[:, 0:1])
        nc.scalar.sqrt(dmy[:, 3:4], dmy[:, 1:2])

        stat_mini_sb = const_pool.tile([MINI_K, 6, MINI_M], mm_dt)
        nc.sync.dma_start(out=stat_mini_sb, in_=stat_mini_dram)
        mit = img_pool.tile([MINI_K, 3, W], mm_dt, tag="mit", bufs=1)
        for c in range(3):
            nc.sync.dma_start(out=mit[:, c, :], in_=img_yx[:, H - 8 : H, c])
        stat_sb = const_pool.tile([TILE_K, 6, STAT_M], mm_dt)
        nc.sync.dma_start(out=stat_sb, in_=stat_dram)

        def epilogue(ps1, ps2, r):
            """f32 x-conv chains reading PSUM directly (one PSUM operand per
            op), magnitude in f32, bf16 only on the final sqrt output.
            Work is balanced: DVE 4 ops, ACT 3 ops + edges, GpSimd 2 ops."""
            gx = work_pool.tile([TILE_K, NW], f32, tag="gx", name="gx")
            gy = work_pool.tile([TILE_K, NW], f32, tag="gy", name="gy")
            _emit_xconv_psum(nc, ps1, gx_taps, gx, r, first_on_act=False)
            _emit_xconv_psum(nc, ps2, gy_taps, gy, r, first_on_act=False)

            sqx = work_pool.tile([TILE_K, NW], f32, tag="sqx", name="sqx")
            nc.gpsimd.tensor_mul(sqx[:r], gx[:r], gx[:r])
            sqy = work_pool.tile([TILE_K, NW], f32, tag="sqy", name="sqy")
            nc.scalar.square(sqy[:r], gy[:r])
            s = work_pool.tile([TILE_K, W], f32, tag="s", name="s")
            nc.gpsimd.tensor_add(s[:r, 1 : 1 + NW], sqx[:r], sqy[:r])
            nc.scalar.copy(s[:r, 0:1], s[:r, 1:2])
            nc.scalar.copy(s[:r, W - 1 : W], s[:r, W - 2 : W - 1])
            mag = work_pool.tile([TILE_K, W], bf16, tag="mag", name="mag")
            nc.scalar.sqrt(mag[:r], s[:r])
            return mag

        # Store triggers go on the Sync ring, delayed by STORE_DELAY tiles so
        # the sqrt-done wait is already satisfied and never blocks the load
        # stream behind it.
        pending_stores = []
        STORE_DELAY = 2

        def flush_stores(upto):
            while pending_stores and len(pending_stores) > upto:
                dst, src = pending_stores.pop(0)
                nc.sync.dma_start(out=dst, in_=src)

        # mini tile first (tiny deps -> PE starts early and ramps its clock)
        mps1 = psum_pool.tile([MINI_M, W], f32, tag="mps1", bufs=1, name="mps1")
        mps2 = psum_pool.tile([MINI_M, W], f32, tag="mps2", bufs=1, name="mps2")
        for gi, ps in ((0, mps1), (1, mps2)):
            for c in range(3):
                nc.tensor.matmul(
                    ps,
                    stat_mini_sb[:, gi * 3 + c, :],
                    mit[:, c, :],
                    start=(c == 0),
                    stop=(c == 2),
                )
        mmag = epilogue(mps1, mps2, MINI_M)
        pending_stores.append((out[:, H - 7 : H, :], mmag[:MINI_M]))

        for n in range(N_PER_CORE):
            for (y0, o0, r, sc0) in BIG_TILES:
                it = img_pool.tile(
                    [TILE_K, 3, W], mm_dt, tag="it", name="it", bufs=5
                )
                nc.sync.dma_start(out=it, in_=img_yx[n, y0 : y0 + TILE_K])
                flush_stores(STORE_DELAY)
                ps1 = psum_pool.tile([STAT_M, W], f32, tag="ps1", name="ps1")
                ps2 = psum_pool.tile([STAT_M, W], f32, tag="ps2", name="ps2")
                for gi, ps in ((0, ps1), (1, ps2)):
                    for c in range(3):
                        nc.tensor.matmul(
                            ps[:r],
                            stat_sb[:, gi * 3 + c, sc0 : sc0 + r],
                            it[:, c, :],
                            start=(c == 0),
                            stop=(c == 2),
                        )
                mag = epilogue(ps1, ps2, r)
                pending_stores.append((out[n, o0 : o0 + r, :], mag[:r]))
        flush_stores(0)


def _build_program_sep(gx_taps, gy_taps):
    import concourse.bacc as bacc
    import concourse.mybir as mybir
    import concourse.tile as tile

    nc = bacc.Bacc(
        "TRN2", target_bir_lowering=False, debug=False, num_devices=N_CORES
    )
    img = nc.dram_tensor(
        "img", [N_PER_CORE, 3, H, W], mybir.dt.float32r, kind="ExternalInput"
    ).ap()
    stat = nc.dram_tensor(
        "stat", [TILE_K, 6, STAT_M], mybir.dt.float32r, kind="ExternalInput"
    ).ap()
    stat_mini = nc.dram_tensor(
        "stat_mini", [MINI_K, 6, MINI_M], mybir.dt.float32r, kind="ExternalInput"
    ).ap()
    out = nc.dram_tensor(
        "out", [N_PER_CORE, H, W], mybir.dt.bfloat16, kind="ExternalOutput"
    ).ap()
    with tile.TileContext(nc) as tc:
        _sobel_body_sep(tc, out, img, stat, stat_mini, gx_taps, gy_taps)
    nc.compile()
    return nc


def _run(nc, in_maps, out_bf16):
    global LAST_RESULTS
    from concourse.bass_utils import run_bass_kernel_spmd

    trace = os.environ.get("SOBEL_TRACE", "0") == "1"
    res = run_bass_kernel_spmd(
        nc, in_maps, core_ids=list(range(N_CORES)), trace=trace
    )
    LAST_RESULTS = res
    outs = [np.asarray(res.results[c]["out"]) for c in range(N_CORES)]
    if out_bf16:
        outs = [o.astype(np.float32) for o in outs]
    out = np.concatenate(outs, axis=0)
    return np.ascontiguousarray(out.reshape(N_FULL, 1, H, W))


def kernel(img: np.ndarray, kx: np.ndarray, ky: np.ndarray) -> np.ndarray:
    img = np.ascontiguousarray(np.asarray(img, dtype=np.float32))
    assert img.shape == (N_FULL, 3, H, W), img.shape

    dx_ = _rank1_decompose(kx) if os.environ.get("SOBEL_NO_SEP", "0") != "1" else None
    dy_ = _rank1_decompose(ky) if dx_ is not None else None
    if dx_ is not None and dy_ is not None:
        (axc, bx, gx_t), (ayc, by, gy_t) = dx_, dy_
        stat, stat_mini = _build_stationaries_sep(axc, bx, ayc, by)
        key = ("sep", tuple(np.round(gx_t, 12)), tuple(np.round(gy_t, 12)))
        if key not in _CACHE:
            _CACHE[key] = _build_program_sep(tuple(gx_t), tuple(gy_t))
        nc = _CACHE[key]
        out_bf16 = True
    else:
        stat, stat_mini = _build_stationaries(kx, ky)
        if "gen" not in _CACHE:
            _CACHE["gen"] = _build_program_gen()
        nc = _CACHE["gen"]
        out_bf16 = False

    in_maps = [
        {
            "img": img[c * N_PER_CORE : (c + 1) * N_PER_CORE],
            "stat": stat,
            "stat_mini": stat_mini,
        }
        for c in range(N_CORES)
    ]
    return _run(nc, in_maps, out_bf16)


# revision 15
# speedup vs baseline: 1.1424x; 1.0799x over previous
"""Sobel filter Trainium2 Bass kernel.

Problem: img [32, 3, 512, 512] f32, kx/ky [1, 3, 3, 3] f32 (same 3x3 kernel
broadcast over the 3 input channels in the reference, but we honor arbitrary
values). Output [32, 1, 512, 512] f32:
    Gx = valid_conv3x3(img, kx), Gy = valid_conv3x3(img, ky)  -> [N,1,510,510]
    out = sqrt(Gx^2 + Gy^2) edge-padded by 1 back to [N,1,512,512]

Strategy (pure data parallel over 8 NeuronCores, 4 images per core):
  The reference Sobel kernels are rank-1 (channel-proportional and separable):
  kG[c, dy, dx] = a[c] * b[dy] * g[dx].  Then
      G = Xconv_g( Sum_c a[c] * Yconv_b(img_c) )
  The y-conv + channel sum runs on the TensorEngine as 3 PSUM-accumulated
  banded matmuls per group (stationary = banded [K=128, M] matrix built from
  a[c]*b on host), the 3-tap x-conv + magnitude runs as a short bf16
  elementwise chain balanced across DVE / ACT / GpSimd.

  Row tiling per image (out rows 0..511):
    tile0:  img rows   0:128 -> out rows   0:127 (127 rows; stationary col 0
            duplicates y'=0 for the top edge pad)
    tile1-3: img rows 126t:126t+128 -> out rows 126t+1 : 126t+127 (126 rows)
    mini:   img rows 504:512 of ALL 4 images ([32, ...] block-diagonal
            stationary) -> out rows 505:512 (7 rows/image; last col of each
            block duplicates y'=509 for the bottom edge pad)
  Each big tile is ONE 3-channel load DMA ([128, 3, 512], 2KB lines) and ONE
  store DMA.  Magnitude is computed and stored in bf16 (rel err ~6e-3, host
  upcasts to f32); this halves store traffic and doubles DVE throughput.

  Epilogue per tile (all work tiles bf16):
    ACT:    cp1 = copy(ps1); sqy = square(gy); edge cols; sqrt; store trigger
    GpSimd: cp2 = copy(ps2); s = sqx + sqy
    DVE:    gx = cp1_0 - cp1_2; gyt = 0.5*cp2_0 + cp2_1;
            gy = 0.5*cp2_2 + gyt; sqx = gx*gx
    Sync:   load triggers
  Both ACT tables (SQUARE, SQRT) are primed by dummy ops at kernel start so
  the lazy table loads don't sit on the critical path.

The general (non-rank-1) fallback keeps the baseline 18-matmul path.
"""

import os

import numpy as np

N_CORES = 8
N_FULL = 32          # full batch
N_PER_CORE = N_FULL // N_CORES
H = W = 512
TILE_K = 128         # input rows per full row-tile
NW = 510             # valid output columns
STAT_M = 127         # stationary cols: dup-edge col + band offsets 0..125
MINI_K = 8 * N_PER_CORE   # 4 images x 8 input rows
MINI_M = 7 * N_PER_CORE   # 4 images x 7 output rows (incl. bottom edge dup)

# big-tile row plan: (img_row0, out_row0, out_rows, stat_col0)
BIG_TILES = [
    (0, 0, 127, 0),
    (126, 127, 126, 1),
    (252, 253, 126, 1),
    (378, 379, 126, 1),
]

_CACHE: dict = {}
LAST_RESULTS = None  # BassKernelResults of the most recent run (for test.py)


# ---------------------------------------------------------------------------
# General fallback (arbitrary kx/ky): banded-Toeplitz matmuls per (g, c, dx).
# Kept from the baseline kernel; only used when the kernels are not rank-1.
# ---------------------------------------------------------------------------

GEN_TILE_M = 126
GEN_N_TILES = 4
GEN_MINI_K = 8 * N_PER_CORE
GEN_MINI_M = 6 * N_PER_CORE


def _build_stationaries(kx: np.ndarray, ky: np.ndarray):
    ks = (np.asarray(kx, np.float32), np.asarray(ky, np.float32))
    stat = np.zeros((18, TILE_K, GEN_TILE_M), np.float32)
    mini = np.zeros((18, GEN_MINI_K, GEN_MINI_M), np.float32)
    m = np.arange(GEN_TILE_M)
    mm = np.arange(6)
    i = 0
    for g in range(2):
        for c in range(3):
            for dx in range(3):
                for dy in range(3):
                    stat[i, m + dy, m] = ks[g][0, c, dy, dx]
                    for j in range(N_PER_CORE):
                        mini[i, j * 8 + mm + dy, j * 6 + mm] = ks[g][0, c, dy, dx]
                i += 1
    return (
        np.ascontiguousarray(stat.transpose(1, 0, 2)),
        np.ascontiguousarray(mini.transpose(1, 0, 2)),
    )


def _epilogue_gen(nc, work_pool, psx, psy, rows, f32):
    s = work_pool.tile([rows, W], f32, tag="s", name="s")
    s2 = work_pool.tile([rows, NW], f32, tag="s2", name="s2")
    nc.scalar.square(s[:, 1 : 1 + NW], psx)
    nc.scalar.square(s2, psy)
    nc.vector.tensor_add(s[:, 1 : 1 + NW], s[:, 1 : 1 + NW], s2)
    nc.vector.tensor_copy(s[:, 0:1], s[:, 1:2])
    nc.vector.tensor_copy(s[:, W - 1 : W], s[:, W - 2 : W - 1])
    mag = work_pool.tile([rows, W], f32, tag="mag", name="mag")
    nc.scalar.sqrt(mag, s)
    return mag


def _sobel_body_gen(tc, out, img, stat_dram, stat_mini_dram):
    import concourse.mybir as mybir

    nc = tc.nc
    f32 = mybir.dt.float32
    mm_dt = mybir.dt.float32r

    img_yx = img.rearrange("n c y x -> n y c x")

    with (
        tc.tile_pool(name="const", bufs=1) as const_pool,
        tc.tile_pool(name="imgs", bufs=3) as img_pool,
        tc.tile_pool(name="work", bufs=4) as work_pool,
        tc.tile_pool(name="psum", bufs=2, space="PSUM") as psum_pool,
    ):
        stat_mini_sb = const_pool.tile([GEN_MINI_K, 18, GEN_MINI_M], mm_dt)
        nc.sync.dma_start(out=stat_mini_sb, in_=stat_mini_dram)
        mit = img_pool.tile([GEN_MINI_K, 3, W], mm_dt, tag="mit", bufs=1)
        for c in range(3):
            nc.sync.dma_start(out=mit[:, c, :], in_=img_yx[:, H - 8 : H, c])
        stat_sb = const_pool.tile([TILE_K, 18, GEN_TILE_M], mm_dt)
        for j in range(5):
            nc.sync.dma_start(
                out=stat_sb[:, 2 * j : 2 * j + 2], in_=stat_dram[:, 2 * j : 2 * j + 2]
            )
        for j in range(5, 9):
            nc.scalar.dma_start(
                out=stat_sb[:, 2 * j : 2 * j + 2], in_=stat_dram[:, 2 * j : 2 * j + 2]
            )

        def big_tile(n, t):
            y0 = t * GEN_TILE_M
            its = []
            for c in range(3):
                itc = img_pool.tile(
                    [TILE_K, W], mm_dt, tag=f"it{c}", name=f"it{c}", bufs=6
                )
                nc.sync.dma_start(out=itc, in_=img_yx[n, y0 : y0 + TILE_K, c])
                its.append(itc)

            psx = psum_pool.tile([GEN_TILE_M, NW], f32, tag="psx", name="psx")
            psy = psum_pool.tile([GEN_TILE_M, NW], f32, tag="psy", name="psy")
            for g, ps in ((0, psx), (1, psy)):
                mmi = 0
                for c in range(3):
                    for dx in range(3):
                        i = (g * 3 + c) * 3 + dx
                        nc.tensor.matmul(
                            ps,
                            stat_sb[:, i, :],
                            its[c][:, dx : dx + NW],
                            start=(mmi == 0),
                            stop=(mmi == 8),
                        )
                        mmi += 1

            mag = _epilogue_gen(nc, work_pool, psx, psy, GEN_TILE_M, f32)
            nc.scalar.dma_start(out=out[n, 1 + y0 : 1 + y0 + GEN_TILE_M, :], in_=mag)
            if t == 0:
                nc.scalar.dma_start(out=out[n, 0:1, :], in_=mag[0:1, :])

        def mini_tile():
            mpsx = psum_pool.tile([GEN_MINI_M, NW], f32, tag="mpsx", bufs=1, name="mpsx")
            mpsy = psum_pool.tile([GEN_MINI_M, NW], f32, tag="mpsy", bufs=1, name="mpsy")
            for g, ps in ((0, mpsx), (1, mpsy)):
                mmi = 0
                for c in range(3):
                    for dx in range(3):
                        i = (g * 3 + c) * 3 + dx
                        nc.tensor.matmul(
                            ps,
                            stat_mini_sb[:, i, :],
                            mit[:, c, dx : dx + NW],
                            start=(mmi == 0),
                            stop=(mmi == 8),
                        )
                        mmi += 1
            mmag = _epilogue_gen(nc, work_pool, mpsx, mpsy, GEN_MINI_M, f32)
            for n in range(N_PER_CORE):
                nc.scalar.dma_start(
                    out=out[n, H - 7 : H - 1, :], in_=mmag[n * 6 : n * 6 + 6]
                )
                nc.scalar.dma_start(
                    out=out[n, H - 1 : H, :], in_=mmag[n * 6 + 5 : n * 6 + 6]
                )

        mini_tile()
        for n in range(N_PER_CORE):
            for t in range(GEN_N_TILES):
                big_tile(n, t)


def _build_program_gen():
    import concourse.bacc as bacc
    import concourse.mybir as mybir
    import concourse.tile as tile

    nc = bacc.Bacc(
        "TRN2",
        target_bir_lowering=False,
        debug=False,
        num_devices=N_CORES,
    )
    img = nc.dram_tensor(
        "img", [N_PER_CORE, 3, H, W], mybir.dt.float32r, kind="ExternalInput"
    ).ap()
    stat = nc.dram_tensor(
        "stat", [TILE_K, 18, GEN_TILE_M], mybir.dt.float32r, kind="ExternalInput"
    ).ap()
    stat_mini = nc.dram_tensor(
        "stat_mini", [GEN_MINI_K, 18, GEN_MINI_M], mybir.dt.float32r,
        kind="ExternalInput",
    ).ap()
    out = nc.dram_tensor(
        "out", [N_PER_CORE, H, W], mybir.dt.float32, kind="ExternalOutput"
    ).ap()

    with tile.TileContext(nc) as tc:
        _sobel_body_gen(tc, out, img, stat, stat_mini)
    nc.compile()
    return nc


# ---------------------------------------------------------------------------
# Separable fast path.
# ---------------------------------------------------------------------------


def _rank1_decompose(k: np.ndarray):
    """k [1,3,3,3] -> (a[3], b[3], g[3]) with k[0,c,dy,dx] = a_c b_dy g_dx,
    or None if not (numerically exactly) rank-1."""
    k2 = np.asarray(k, np.float64)[0]
    scale = np.abs(k2).max()
    if scale == 0:
        return None
    u, s, vt = np.linalg.svd(k2.reshape(3, 9), full_matrices=False)
    a = u[:, 0] * s[0]
    v = vt[0].reshape(3, 3)
    u2, s2, vt2 = np.linalg.svd(v, full_matrices=False)
    b = u2[:, 0] * s2[0]
    g = vt2[0]
    rec = np.einsum("c,y,x->cyx", a, b, g)
    if np.abs(rec - k2).max() > 1e-6 * scale:
        return None
    # normalize so the largest |g| tap is exactly 1
    gm = g[np.argmax(np.abs(g))]
    g = g / gm
    a = a * gm
    return a.astype(np.float64), b.astype(np.float64), g.astype(np.float64)


def _build_stationaries_sep(ax, bx, ay, by):
    """stat_sep [TILE_K, 6, STAT_M] (j = G*3+c): col 0 duplicates the y'=0
    band (top edge pad), cols 1..126 are band offsets 0..125.
    stat_sep_mini [MINI_K, 6, MINI_M]: block-diagonal per image, 7 out cols
    each (col 6 duplicates y'=509 for the bottom edge pad)."""
    stat = np.zeros((6, TILE_K, STAT_M), np.float32)
    mini = np.zeros((6, MINI_K, MINI_M), np.float32)
    m = np.arange(126)
    mm = np.arange(6)
    for gi, (a, b) in enumerate(((ax, bx), (ay, by))):
        for c in range(3):
            j = gi * 3 + c
            for dy in range(3):
                w = np.float32(a[c] * b[dy])
                stat[j, dy, 0] = w              # dup col: y'=0
                stat[j, m + dy, m + 1] = w      # band offsets 0..125
                for im in range(N_PER_CORE):
                    mini[j, im * 8 + mm + dy, im * 7 + mm] = w
                    mini[j, im * 8 + 5 + dy, im * 7 + 6] = w  # dup col: y'=509
    return (
        np.ascontiguousarray(stat.transpose(1, 0, 2)),
        np.ascontiguousarray(mini.transpose(1, 0, 2)),
    )


def _emit_xconv_psum(nc, ps, taps, outt, r, first_on_act=False):
    """outt[:r] = x-conv of PSUM tile ps[:r] with 3-tap `taps`, chained
    in place (each op reads one PSUM slice + the SBUF accumulator).
    Optionally the first (scale-copy) op goes on ACT to offload DVE."""
    import concourse.mybir as mybir

    L = [(float(taps[dx]), dx) for dx in range(3) if taps[dx] != 0.0]
    assert L
    mult, add = mybir.AluOpType.mult, mybir.AluOpType.add
    acc = None
    for w, dx in L:
        src = ps[:r, dx : dx + NW]
        if acc is None:
            if first_on_act and len(L) > 1:
                # ACT scale-copy via the Identity func path (Copy's scale
                # path produces wrong results on HW).
                nc.scalar.activation(
                    outt[:r], src, mybir.ActivationFunctionType.Identity,
                    bias=0.0, scale=w,
                )
            else:
                nc.vector.tensor_scalar_mul(outt[:r], src, w)
        else:
            nc.vector.scalar_tensor_tensor(outt[:r], src, w, acc[:r], mult, add)
        acc = outt


def _sobel_body_sep(tc, out, img, stat_dram, stat_mini_dram, gx_taps, gy_taps):
    import concourse.mybir as mybir

    nc = tc.nc
    f32 = mybir.dt.float32
    bf16 = mybir.dt.bfloat16
    mm_dt = mybir.dt.float32r

    img_yx = img.rearrange("n c y x -> n y c x")

    with (
        tc.tile_pool(name="const", bufs=1) as const_pool,
        tc.tile_pool(name="imgs", bufs=5) as img_pool,
        tc.tile_pool(name="work", bufs=3) as work_pool,
        tc.tile_pool(name="psum", bufs=3, space="PSUM") as psum_pool,
    ):
        # Prime both ACT tables (SQUARE, SQRT) while the first DMAs stream so
        # the 1.3us lazy table loads stay off the critical path.
        dmy = const_pool.tile([1, 4], f32)
        nc.vector.memset(dmy[:, 0:2], 1.0)
        nc.scalar.square(dmy[:, 2:3], dmy[:, 0:1])
        nc.scalar.sqrt(dmy[:, 3:4], dmy[:, 1:2])

        stat_mini_sb = const_pool.tile([MINI_K, 6, MINI_M], mm_dt)
        nc.sync.dma_start(out=stat_mini_sb, in_=stat_mini_dram)
        mit = img_pool.tile([MINI_K, 3, W], mm_dt, tag="mit", bufs=1)
        for c in range(3):
            nc.sync.dma_start(out=mit[:, c, :], in_=img_yx[:, H - 8 : H, c])
        stat_sb = const_pool.tile([TILE_K, 6, STAT_M], mm_dt)
        nc.sync.dma_start(out=stat_sb, in_=stat_dram)

        def epilogue(ps1, ps2, r):
            """f32 x-conv chains reading PSUM directly (one PSUM operand per
            op), magnitude in f32, bf16 only on the final sqrt output.
            Work is balanced: DVE 4 ops, ACT 3 ops + edges, GpSimd 2 ops."""
            gx = work_pool.tile([TILE_K, NW], f32, tag="gx", name="gx")
            gy = work_pool.tile([TILE_K, NW], f32, tag="gy", name="gy")
            _emit_xconv_psum(nc, ps1, gx_taps, gx, r, first_on_act=False)
            _emit_xconv_psum(nc, ps2, gy_taps, gy, r, first_on_act=True)

            sqx = work_pool.tile([TILE_K, NW], f32, tag="sqx", name="sqx")
            nc.gpsimd.tensor_mul(sqx[:r], gx[:r], gx[:r])
            sqy = work_pool.tile([TILE_K, NW], f32, tag="sqy", name="sqy")
            nc.scalar.square(sqy[:r], gy[:r])
            s = work_pool.tile([TILE_K, W], f32, tag="s", name="s")
            nc.gpsimd.tensor_add(s[:r, 1 : 1 + NW], sqx[:r], sqy[:r])
            nc.scalar.copy(s[:r, 0:1], s[:r, 1:2])
            nc.scalar.copy(s[:r, W - 1 : W], s[:r, W - 2 : W - 1])
            mag = work_pool.tile([TILE_K, W], bf16, tag="mag", name="mag", bufs=7)
            nc.scalar.sqrt(mag[:r], s[:r])
            return mag

        # Store triggers go on the Sync ring, delayed by STORE_DELAY tiles so
        # the sqrt-done wait is long satisfied when the trigger issues and
        # never blocks the load stream behind it.
        pending_stores = []
        STORE_DELAY = 4

        def flush_stores(upto):
            while pending_stores and len(pending_stores) > upto:
                dst, src = pending_stores.pop(0)
                nc.sync.dma_start(out=dst, in_=src)

        # mini tile first (tiny deps -> PE starts early and ramps its clock)
        mps1 = psum_pool.tile([MINI_M, W], f32, tag="mps1", bufs=1, name="mps1")
        mps2 = psum_pool.tile([MINI_M, W], f32, tag="mps2", bufs=1, name="mps2")
        for gi, ps in ((0, mps1), (1, mps2)):
            for c in range(3):
                nc.tensor.matmul(
                    ps,
                    stat_mini_sb[:, gi * 3 + c, :],
                    mit[:, c, :],
                    start=(c == 0),
                    stop=(c == 2),
                )
        mmag = epilogue(mps1, mps2, MINI_M)
        pending_stores.append((out[:, H - 7 : H, :], mmag[:MINI_M]))

        for n in range(N_PER_CORE):
            for (y0, o0, r, sc0) in BIG_TILES:
                it = img_pool.tile(
                    [TILE_K, 3, W], mm_dt, tag="it", name="it", bufs=5
                )
                nc.sync.dma_start(out=it, in_=img_yx[n, y0 : y0 + TILE_K])
                ps1 = psum_pool.tile([STAT_M, W], f32, tag="ps1", name="ps1")
                ps2 = psum_pool.tile([STAT_M, W], f32, tag="ps2", name="ps2")
                for gi, ps in ((0, ps1), (1, ps2)):
                    for c in range(3):
                        nc.tensor.matmul(
                            ps[:r],
                            stat_sb[:, gi * 3 + c, sc0 : sc0 + r],
                            it[:, c, :],
                            start=(c == 0),
                            stop=(c == 2),
                        )
                flush_stores(STORE_DELAY)
                mag = epilogue(ps1, ps2, r)
                pending_stores.append((out[n, o0 : o0 + r, :], mag[:r]))
        flush_stores(0)


def _build_program_sep(gx_taps, gy_taps):
    import concourse.bacc as bacc
    import concourse.mybir as mybir
    import concourse.tile as tile

    nc = bacc.Bacc(
        "TRN2", target_bir_lowering=False, debug=False, num_devices=N_CORES
    )
    img = nc.dram_tensor(
        "img", [N_PER_CORE, 3, H, W], mybir.dt.float32r, kind="ExternalInput"
    ).ap()
    stat = nc.dram_tensor(
        "stat", [TILE_K, 6, STAT_M], mybir.dt.float32r, kind="ExternalInput"
    ).ap()
    stat_mini = nc.dram_tensor(
        "stat_mini", [MINI_K, 6, MINI_M], mybir.dt.float32r, kind="ExternalInput"
    ).ap()
    out = nc.dram_tensor(
        "out", [N_PER_CORE, H, W], mybir.dt.bfloat16, kind="ExternalOutput"
    ).ap()
    with tile.TileContext(nc) as tc:
        _sobel_body_sep(tc, out, img, stat, stat_mini, gx_taps, gy_taps)
    nc.compile()
    return nc


def _run(nc, in_maps, out_bf16):
    global LAST_RESULTS
    from concourse.bass_utils import run_bass_kernel_spmd

    trace = os.environ.get("SOBEL_TRACE", "0") == "1"
    res = run_bass_kernel_spmd(
        nc, in_maps, core_ids=list(range(N_CORES)), trace=trace
    )
    LAST_RESULTS = res
    outs = [np.asarray(res.results[c]["out"]) for c in range(N_CORES)]
    if out_bf16:
        outs = [o.astype(np.float32) for o in outs]
    out = np.concatenate(outs, axis=0)
    return np.ascontiguousarray(out.reshape(N_FULL, 1, H, W))


def kernel(img: np.ndarray, kx: np.ndarray, ky: np.ndarray) -> np.ndarray:
    img = np.ascontiguousarray(np.asarray(img, dtype=np.float32))
    assert img.shape == (N_FULL, 3, H, W), img.shape

    dx_ = _rank1_decompose(kx) if os.environ.get("SOBEL_NO_SEP", "0") != "1" else None
    dy_ = _rank1_decompose(ky) if dx_ is not None else None
    if dx_ is not None and dy_ is not None:
        (axc, bx, gx_t), (ayc, by, gy_t) = dx_, dy_
        stat, stat_mini = _build_stationaries_sep(axc, bx, ayc, by)
        key = ("sep", tuple(np.round(gx_t, 12)), tuple(np.round(gy_t, 12)))
        if key not in _CACHE:
            _CACHE[key] = _build_program_sep(tuple(gx_t), tuple(gy_t))
        nc = _CACHE[key]
        out_bf16 = True
    else:
        stat, stat_mini = _build_stationaries(kx, ky)
        if "gen" not in _CACHE:
            _CACHE["gen"] = _build_program_gen()
        nc = _CACHE["gen"]
        out_bf16 = False

    in_maps = [
        {
            "img": img[c * N_PER_CORE : (c + 1) * N_PER_CORE],
            "stat": stat,
            "stat_mini": stat_mini,
        }
        for c in range(N_CORES)
    ]
    return _run(nc, in_maps, out_bf16)
